# revision 1
# baseline (speedup 1.0000x reference)
"""CommutatorConv2d kernel for Trainium2 (Bass/Tile), 8-core data-parallel.

Math: the reference's commutator/anticommutator conv reduces exactly to a
single-channel 3x3 conv on the channel-summed input:

    out[b] = T @ xs[b] @ A + Bm @ xs[b] @ T + bias,   xs = x.sum(axis=1)

where T is the 128x128 tridiagonal-ones matrix and A, Bm are tridiagonal
matrices built from K's column/row sums scaled by (lambda_c +/- lambda_a):
sum_{i,m} XK[...,i,m] = sum_{i,j} patch[i,j]*colsum(K)[j] and
sum_{j,i} KX[...,j,i] = sum_{m,i} patch[m,i]*rowsum(K)[m], so the effective
3x3 kernel is W[i,j] = a[j] + b[i], separable into a row-conv on the vertical
boxsum plus a col-conv on the horizontal boxsum = the two matrix sandwiches.

Layout: each core's batch shard is handed to the device as [H, B_loc, C, W]
(h-major) so every SBUF partition receives one long contiguous DRAM run per
DMA — 8KB descriptors instead of 512B ones, which is the difference between
~170 GB/s and ~358 GB/s on the HBM path. The device still streams the full
shard HBM->SBUF.

Per core (2 batches x 4 pieces of 8 channels): the channel fold is split
between the vector engine (in-place contiguous binary-tree adds over the
early-arriving pieces — hidden under the DMA window) and the tensor engine
(identity-matmul PSUM accumulation over the late pieces — the shorter
post-DMA dependency chain). Then uv = xs.T @ [T | Bm.T] (one K=128,N=256
matmul), out = uv[:, :128].T @ A + uv[:, 128:].T @ T accumulated in PSUM,
bias-add on the scalar engine into a combined output tile, and one 1KB-run
store on the SWDGE path. x pieces ride the sync HWDGE ring; the fused
constant matrix rides the scalar ring so the identity lands first.
"""

import numpy as np

B, C, H, W = 16, 32, 128, 128
N_CORES = 8
B_LOC = B // N_CORES

_PROGRAM = None
LAST_RESULTS = None


def _build_program():
    import concourse.mybir as mybir
    from concourse import bacc
    from concourse.bass import MemorySpace
    from concourse.tile import TileContext

    f32 = mybir.dt.float32
    nc = bacc.Bacc(
        "TRN2", target_bir_lowering=False, debug=False, num_devices=N_CORES
    )

    x_dram = nc.dram_tensor("x", (H, B_LOC, C, W), f32, kind="ExternalInput")
    # fused constants: [A | T | TBm | I | bias_col] as columns
    cm_dram = nc.dram_tensor("cmat", (H, 5 * W + 1), f32, kind="ExternalInput")
    # h-major output (host transposes back) -> 1KB contiguous runs per
    # partition and a single store
    out_dram = nc.dram_tensor("out", (H, B_LOC, W), f32, kind="ExternalOutput")

    x_ap = x_dram.ap()
    out_ap = out_dram.ap()

    with TileContext(nc) as tc:
        with (
            tc.tile_pool(name="consts", bufs=1) as cpool,
            tc.tile_pool(name="xpool", bufs=3) as xpool,
            tc.tile_pool(name="uvpool", bufs=2) as uvpool,
            tc.tile_pool(name="opool", bufs=2) as opool,
            tc.tile_pool(name="psum", bufs=2, space=MemorySpace.PSUM) as ppool,
        ):
            # Fused constants on the otherwise-idle scalar HWDGE ring so the
            # identity matrix lands before the first x piece does.
            cm_sb = cpool.tile([H, 5 * W + 1], f32)
            nc.scalar.dma_start(out=cm_sb, in_=cm_dram.ap())
            a_sb = cm_sb[:, 0:W]
            t_sb = cm_sb[:, W : 2 * W]
            tbm_sb = cm_sb[:, 2 * W : 4 * W]
            i_sb = cm_sb[:, 4 * W : 5 * W]
            bias_sb = cm_sb[:, 5 * W : 5 * W + 1]

            # x streams in 8-channel pieces (1024 free elems = 4KB runs per
            # partition, sync HWDGE ring). Per batch: pieces 0-1 fold on the
            # tensor engine (identity-matmul PSUM accumulation), pieces 2-3
            # fold on the vector engine (in-place binary tree). The fold work
            # is split so BOTH engines fit inside the DMA streaming window,
            # and each batch ends on a DVE piece for the shortest tail.
            PIECE = 8  # channels per DMA piece
            PIECES = C // PIECE  # 4
            o2_sb = opool.tile([H, B_LOC * W], f32)
            xs_list = []
            for b in range(B_LOC):
                # Each batch splits its fold between DVE trees (early pieces,
                # hidden under the DMA window) and PE identity-quads (late
                # pieces — the post-DMA chain through the tensor engine is
                # the shorter one).
                use_pe = True
                tiles = {}
                # DVE pieces stream first: their trees consume tiles early,
                # keeping the tile-slot recycling smooth for the next batch
                # (PE-piece-first ordering stalls the DMA ring on slot reuse).
                for p in (2, 3, 0, 1):
                    xq = xpool.tile([H, PIECE * W], f32, tag=f"xq{p}")
                    nc.sync.dma_start(
                        out=xq.rearrange("h (c w) -> h c w", w=W),
                        in_=x_ap[:, b, p * PIECE : (p + 1) * PIECE, :],
                    )
                    tiles[p] = xq

                tree_pieces = (2, 3) if use_pe else (2, 3, 0, 1)
                for p in tree_pieces:
                    xq = tiles[p]
                    n = PIECE * W
                    while n > W:
                        n //= 2
                        nc.vector.tensor_add(xq[:, :n], xq[:, :n], xq[:, n : 2 * n])
                nc.vector.tensor_add(
                    tiles[2][:, :W], tiles[2][:, :W], tiles[3][:, :W]
                )
                xs = tiles[2][:, :W]

                if use_pe:
                    # PE fold of pieces 0-1: cs_psum accumulates four
                    # 4-channel groups elementwise -> [H, 4, W] partials
                    cs_psum = ppool.tile([H, 4 * W], f32)
                    q = 0
                    for p in range(2):
                        for half in range(2):
                            nc.tensor.matmul(
                                cs_psum,
                                i_sb,
                                tiles[p][:, half * 4 * W : (half + 1) * 4 * W],
                                start=(q == 0),
                                stop=(q == 3),
                            )
                            q += 1
                    cs_sb = uvpool.tile([H, 4 * W], f32, tag="cs")
                    nc.vector.tensor_copy(cs_sb, cs_psum)
                    nc.vector.tensor_add(
                        cs_sb[:, : 2 * W],
                        cs_sb[:, : 2 * W],
                        cs_sb[:, 2 * W : 4 * W],
                    )
                    nc.vector.tensor_add(
                        cs_sb[:, :W], cs_sb[:, :W], cs_sb[:, W : 2 * W]
                    )
                    nc.vector.tensor_add(xs, xs, cs_sb[:, :W])
                else:
                    nc.vector.tensor_add(
                        tiles[0][:, :W], tiles[0][:, :W], tiles[1][:, :W]
                    )
                    nc.vector.tensor_add(xs, xs, tiles[0][:, :W])
                xs_list.append(xs)

            # Phase 2: matmul chains for all batches AFTER all folds are
            # emitted, so the last batch's quads aren't queued behind the
            # first batch's uv-copy-gated stage-2 on the in-order PE queue.
            for b in range(B_LOC):
                xs = xs_list[b]
                uv_psum = ppool.tile([H, 2 * W], f32)
                nc.tensor.matmul(uv_psum, xs, tbm_sb, start=True, stop=True)
                uv_sb = uvpool.tile([H, 2 * W], f32)
                # split copies: stage-2's first matmul starts after half
                nc.vector.tensor_copy(uv_sb[:, 0:W], uv_psum[:, 0:W])
                nc.vector.tensor_copy(uv_sb[:, W : 2 * W], uv_psum[:, W : 2 * W])

                o_psum = ppool.tile([H, W], f32)
                nc.tensor.matmul(o_psum, uv_sb[:, 0:W], a_sb, start=True, stop=False)
                nc.tensor.matmul(
                    o_psum, uv_sb[:, W : 2 * W], t_sb, start=False, stop=True
                )

                # bias-add rides the idle scalar engine, off the DVE queue
                nc.scalar.add(o2_sb[:, b * W : (b + 1) * W], o_psum, add=bias_sb)

            # one store, 1KB runs per partition, on the sync HWDGE ring
            # (idle after the loads; ~0.6us first-byte vs ~1us on SWDGE)
            nc.sync.dma_start(
                out=out_ap, in_=o2_sb.rearrange("h (b w) -> h b w", w=W)
            )

    nc.compile()
    return nc


def _get_program():
    global _PROGRAM
    if _PROGRAM is None:
        _PROGRAM = _build_program()
    return _PROGRAM


def _build_consts(K, bias, lambda_c, lambda_a):
    K = np.asarray(K, np.float32)
    lc = float(np.asarray(lambda_c))
    la = float(np.asarray(lambda_a))
    a = (lc + la) * K.sum(axis=0)  # column sums -> horizontal taps
    b = (la - lc) * K.sum(axis=1)  # row sums -> vertical taps
    eye = np.eye(H, dtype=np.float32)
    up = np.eye(H, k=1, dtype=np.float32)
    dn = np.eye(H, k=-1, dtype=np.float32)
    T = eye + up + dn
    A = a[1] * eye + a[0] * up + a[2] * dn
    Bm = b[1] * eye + b[2] * up + b[0] * dn
    bias_col = np.full((H, 1), np.asarray(bias, np.float32).reshape(-1)[0], np.float32)
    # fused [A | T | T | Bm.T | I | bias_col] -> [H, 5W+1]
    cm = np.concatenate([A, T, T, Bm.T, eye, bias_col], axis=1)
    return np.ascontiguousarray(cm, np.float32)


def kernel(x, K, bias, lambda_c, lambda_a, _trace=False):
    global LAST_RESULTS
    from concourse.bass_utils import run_bass_kernel_spmd

    x = np.asarray(x, np.float32)
    cm = _build_consts(K, bias, lambda_c, lambda_a)
    nc = _get_program()

    in_maps = []
    for core in range(N_CORES):
        shard = x[core * B_LOC : (core + 1) * B_LOC]  # [B_LOC, C, H, W]
        shard_t = np.ascontiguousarray(shard.transpose(2, 0, 1, 3))  # [H,B,C,W]
        in_maps.append({"x": shard_t, "cmat": cm})

    res = run_bass_kernel_spmd(
        nc, in_maps, core_ids=list(range(N_CORES)), trace=_trace
    )
    LAST_RESULTS = res
    # per-core outputs are [H, B_LOC, W]; swap back to [B_LOC, H, W]
    out = np.concatenate(
        [r["out"].transpose(1, 0, 2) for r in res.results], axis=0
    )
    return out.reshape(B, 1, H, W).astype(np.float32, copy=False)



# revision 12
# speedup vs baseline: 1.3390x; 1.3390x over previous
"""CommutatorConv2d kernel for Trainium2 (Bass/Tile), 8-core data-parallel.

Math: the reference's commutator/anticommutator conv reduces exactly to a
single-channel 3x3 conv on the channel-summed input:

    out[b] = T @ xs[b] @ A + Bm @ xs[b] @ T + bias,   xs = x.sum(axis=1)

where T is the 128x128 tridiagonal-ones matrix and A, Bm are tridiagonal
matrices built from K's column/row sums scaled by (lambda_c +/- lambda_a):
sum_{i,m} XK[...,i,m] = sum_{i,j} patch[i,j]*colsum(K)[j] and
sum_{j,i} KX[...,j,i] = sum_{m,i} patch[m,i]*rowsum(K)[m], so the effective
3x3 kernel is W[i,j] = a[j] + b[i], separable into a row-conv on the vertical
boxsum plus a col-conv on the horizontal boxsum = the two matrix sandwiches.

Precision: x and the constant matrices are cast to bf16 on the host. The
kernel is one long summation (channel fold + two tridiagonal sandwiches)
whose accumulations all happen in fp32 PSUM; only the element roundings are
bf16, giving ~4e-3 relative error against the fp32 reference — an order of
magnitude inside the 2e-2 gate — while halving HBM traffic (the kernel is
HBM-bound) and doubling PE/DVE throughput.

Layout: each core's shard is sent as [H, B_loc, C, W] (h-major) so every
SBUF partition receives one contiguous 2KB DRAM run per 8-channel piece.

Schedule: all 8 x-piece DMAs are issued up-front, alternating between the
sync and scalar HWDGE queues, so the HBM path streams gap-free. Per batch,
pieces 0-1 fold on the tensor engine as bf16 identity-matmul quads into one
fp32 PSUM accumulator; pieces 2-3 fold on the vector engine as in-place
binary trees (bf16 runs DVE at 2x). The partial sums are never merged into
one xs: uv accumulates all three (PSUM quad-fold + two tree results) with
back-to-back matmuls against [T | Bm.T]. The activation engine does every
PSUM evacuation (quad partial, split uv halves, final bias-add), keeping
the vector engine exclusively on the fold. Stage 2 multiplies the uv halves
by A and T, bias adds straight out of PSUM in fp32, and each batch's [H, W]
fp32 result stores separately on the sync queue (batch 0 mid-stream).
"""

import numpy as np

B, C, H, W = 16, 32, 128, 128
N_CORES = 8
B_LOC = B // N_CORES

_PROGRAM = None
LAST_RESULTS = None


def _build_program():
    import concourse.mybir as mybir
    from concourse import bacc
    from concourse.bass import MemorySpace
    from concourse.tile import TileContext

    f32 = mybir.dt.float32
    bf16 = mybir.dt.bfloat16
    nc = bacc.Bacc(
        "TRN2", target_bir_lowering=False, debug=False, num_devices=N_CORES
    )

    x_dram = nc.dram_tensor("x", (H, B_LOC, C, W), bf16, kind="ExternalInput")
    # fused constants: [T | Bm.T | A | I | bias] as bf16 columns; the last
    # two columns hold each partition's fp32 bias value as raw bits
    cm_dram = nc.dram_tensor("cmat", (H, 4 * W + 2), bf16, kind="ExternalInput")
    out_dram = nc.dram_tensor("out", (H, B_LOC, W), f32, kind="ExternalOutput")

    x_ap = x_dram.ap()
    out_ap = out_dram.ap()

    PIECE = 8  # channels per DMA piece
    PE_PIECES = (0, 1)  # fold on tensor engine (identity quads)
    DVE_PIECES = (2, 3)  # fold on vector engine (binary tree)

    with TileContext(nc) as tc:
        with (
            tc.tile_pool(name="consts", bufs=1) as cpool,
            tc.tile_pool(name="xpool", bufs=2) as xpool,
            tc.tile_pool(name="spool", bufs=2) as spool,
            tc.tile_pool(name="psum", bufs=2, space=MemorySpace.PSUM) as ppool,
        ):
            cm_sb = cpool.tile([H, 4 * W + 2], bf16)
            t_sb = cm_sb[:, 0:W]
            tbm_sb = cm_sb[:, 0 : 2 * W]
            a_sb = cm_sb[:, 2 * W : 3 * W]
            i_sb = cm_sb[:, 3 * W : 4 * W]
            bias_sb = cm_sb[:, 4 * W : 4 * W + 2].bitcast(f32)

            # ---- phase 0: every load issued up-front across both HWDGE
            # queues; nothing on the issue path waits on anything.
            nc.scalar.dma_start(out=cm_sb, in_=cm_dram.ap())
            tiles = {}
            order = [(0, 0), (0, 1), (0, 2), (0, 3), (1, 0), (1, 1), (1, 2), (1, 3)]
            for idx, (b, p) in enumerate(order):
                xq = xpool.tile([H, PIECE * W], bf16, tag=f"xq{p}")
                eng = nc.sync if idx % 2 == 0 else nc.scalar
                eng.dma_start(
                    out=xq.rearrange("h (c w) -> h c w", w=W),
                    in_=x_ap[:, b, p * PIECE : (p + 1) * PIECE, :],
                )
                tiles[(b, p)] = xq

            # ---- phase 1, per batch. Per-engine queue orders that fall out
            # (each engine dispatches in-order; deps resolve monotonically):
            #   PE:  b quads(4) -> b uv(3) -> b stage2(2) -> next batch
            #   DVE: b cs-folds(2) -> b p2 tree(3) -> b p3 tree(3) -> next
            #   ACT: b cs-copy -> b uv copies(2) -> b bias -> next
            #   sync: 4 x-loads -> b0 store -> b1 store
            for b in range(B_LOC):
                # PE: 4 identity quads fold pieces 0-1 into fp32 PSUM
                cs_psum = ppool.tile([H, 4 * W], f32, tag="csp")
                q = 0
                for p in PE_PIECES:
                    for half in range(2):
                        nc.tensor.matmul(
                            cs_psum,
                            i_sb,
                            tiles[(b, p)][:, half * 4 * W : (half + 1) * 4 * W],
                            start=(q == 0),
                            stop=(q == 3),
                        )
                        q += 1
                # ACT evacuates the 4-way partial (rounding to bf16)
                cs = spool.tile([H, 4 * W], bf16, tag="cs")
                nc.scalar.copy(cs, cs_psum)

                # DVE: fold the quad partial, then tree-fold pieces 2-3
                # in-place. The p3 tree is emitted last so it is the only
                # fold ahead of the final piece when the stream ends.
                nc.vector.tensor_add(
                    cs[:, 0 : 2 * W], cs[:, 0 : 2 * W], cs[:, 2 * W : 4 * W]
                )
                nc.vector.tensor_add(cs[:, 0:W], cs[:, 0:W], cs[:, W : 2 * W])
                for p in DVE_PIECES:
                    xq = tiles[(b, p)]
                    n = PIECE * W
                    while n > W:
                        n //= 2
                        nc.vector.tensor_add(xq[:, :n], xq[:, :n], xq[:, n : 2 * n])

                # PE: uv accumulates all three partial sums against [T|Bm.T]
                uv_psum = ppool.tile([H, 2 * W], f32, tag="uvp")
                nc.tensor.matmul(uv_psum, cs[:, 0:W], tbm_sb, start=True, stop=False)
                nc.tensor.matmul(
                    uv_psum, tiles[(b, 2)][:, 0:W], tbm_sb, start=False, stop=False
                )
                nc.tensor.matmul(
                    uv_psum, tiles[(b, 3)][:, 0:W], tbm_sb, start=False, stop=True
                )
                uv = spool.tile([H, 2 * W], bf16, tag="uv")
                # split copies so stage-2's first matmul starts after half
                nc.scalar.copy(uv[:, 0:W], uv_psum[:, 0:W])
                nc.scalar.copy(uv[:, W : 2 * W], uv_psum[:, W : 2 * W])

                op = ppool.tile([H, W], f32, tag="op")
                nc.tensor.matmul(op, uv[:, 0:W], a_sb, start=True, stop=False)
                nc.tensor.matmul(op, uv[:, W : 2 * W], t_sb, start=False, stop=True)

                o2b = spool.tile([H, W], f32, tag="o2")
                nc.scalar.add(o2b, op, add=bias_sb)
                # batch-0 stores mid-stream, batch-1 at the end; both on the
                # sync queue whose loads have drained by then
                nc.sync.dma_start(out=out_ap[:, b, :], in_=o2b)

    nc.compile()
    return nc


def _get_program():
    global _PROGRAM
    if _PROGRAM is None:
        _PROGRAM = _build_program()
    return _PROGRAM


def _build_consts(K, bias, lambda_c, lambda_a, np_bf16):
    K = np.asarray(K, np.float32)
    lc = float(np.asarray(lambda_c))
    la = float(np.asarray(lambda_a))
    a = (lc + la) * K.sum(axis=0)  # column sums -> horizontal taps
    b = (la - lc) * K.sum(axis=1)  # row sums -> vertical taps
    eye = np.eye(H, dtype=np.float32)
    up = np.eye(H, k=1, dtype=np.float32)
    dn = np.eye(H, k=-1, dtype=np.float32)
    T = eye + up + dn
    A = a[1] * eye + a[0] * up + a[2] * dn
    Bm = b[1] * eye + b[2] * up + b[0] * dn
    # fused [T | Bm.T | A | I] in bf16, then the fp32 bias bit-packed into
    # two trailing bf16 columns
    cm = np.concatenate([T, Bm.T, A, eye], axis=1).astype(np_bf16)
    bias_col = np.full(
        (H, 1), np.asarray(bias, np.float32).reshape(-1)[0], np.float32
    )
    bias_bits = bias_col.view(np.uint16).view(np_bf16)  # [H, 2] raw halves
    return np.ascontiguousarray(np.concatenate([cm, bias_bits], axis=1))


def kernel(x, K, bias, lambda_c, lambda_a, _trace=False):
    global LAST_RESULTS
    import concourse.mybir as mybir
    from concourse.bass_utils import run_bass_kernel_spmd

    np_bf16 = mybir.dt.np(mybir.dt.bfloat16)
    x = np.asarray(x, np.float32)
    cm = _build_consts(K, bias, lambda_c, lambda_a, np_bf16)
    nc = _get_program()

    in_maps = []
    for core in range(N_CORES):
        shard = x[core * B_LOC : (core + 1) * B_LOC]  # [B_LOC, C, H, W]
        shard_t = np.ascontiguousarray(
            shard.transpose(2, 0, 1, 3).astype(np_bf16)
        )  # [H, B, C, W] bf16
        in_maps.append({"x": shard_t, "cmat": cm})

    res = run_bass_kernel_spmd(
        nc, in_maps, core_ids=list(range(N_CORES)), trace=_trace
    )
    LAST_RESULTS = res
    # per-core outputs are [H, B_LOC, W]; swap back to [B_LOC, H, W]
    out = np.concatenate(
        [r["out"].transpose(1, 0, 2) for r in res.results], axis=0
    )
    return out.reshape(B, 1, H, W).astype(np.float32, copy=False)
